# revision 8
# baseline (speedup 1.0000x reference)
"""Trainium2 Bass kernel for nn_DIGQ_91319594648354 (gnn_message_passing).

Algorithm notes (validated vs reference in fp32 to ~6e-7 rel):
- In the reference, softmax over j of (si_i + sj_j + ss + attn_b) reduces to
  softmax_j(sj_j): everything constant in j cancels. So attention weights are
  independent of i, the state encoder / Wi / Ws / attn_b contribute nothing,
  and messages are a single per-head weighted mean broadcast to all agents.
- All additive biases (node_b, upd_b2) fold host-side into per-layer effective
  biases (b1eff_l, qp_b1eff); device activations track h_tilde = h - bcum.

Device layout (per core; 512 batch elements = 4096 agent-rows):
- Spine: transposed activations [feature(128-part), row(4096-free)], all
  weight-stationary matmuls (out = W.T-form @ actT).
- Sidecar per layer: PE transpose pass (lhsT=hT chunk, rhs=I128) yields
  row-major h chunks + sj (rhs=Wj rides the same stationary); softmax via
  block-ones matmuls (j-group sums over partition groups of 8); block-diagonal
  softmax weights built by one masked DVE multiply per chunk; msg matmul
  (lhsT=h_rm chunk, rhs=wbd) gives msgT[e,(g,h)]; m2T accumulates 4 per-head
  W1bot matmuls.
Data parallel over 8 cores; weights replicated.
"""
import os
import sys

import numpy as np

for _p in ("/opt/trn_rl_repo",):
    if _p not in sys.path and os.path.isdir(_p):
        sys.path.insert(0, _p)

import concourse.bass as bass
import concourse.bacc as bacc
import concourse.mybir as mybir
from concourse import bass_utils
from concourse.tile import TileContext
from contextlib import ExitStack

BS, SEQ, N, OBS, SD, E, H, L = 16, 256, 8, 64, 128, 128, 4, 2
B = BS * SEQ            # 4096
NCORES = 8
BC = B // NCORES        # 512 elements per core
R = BC * N              # 4096 rows per core
NCH = R // 128          # 32 chunks of 128 rows
GPC = 128 // N          # 16 elements per chunk

F32 = mybir.dt.float32
AF = mybir.ActivationFunctionType

# blob column map
_BI = 0            # I128            [128, 128]
_BONES = 128       # bones           [128, 16]   bones[r,g] = (r//8==g)
_BONEST = 144      # bonesT          [16, 128]   transpose of bones
_BMASK = 272       # mask64          [128, 64]   mask[r, g*4+h] = (r//8==g)
_BWJ0 = 336        # Wj layer0       [128, 4]
_BWJ1 = 340        # Wj layer1       [128, 4]
_BB1E0 = 344       # b1eff layer0    [128, 1]
_BB1E1 = 345       # b1eff layer1    [128, 1]
_BQPB1 = 346       # qp_b1eff        [128, 1]
_BQPB2 = 347       # qp_b2           [1, 1]
_BLOBC = 348


def _build_program():
    nc = bacc.Bacc("TRN2", target_bir_lowering=False, debug=False,
                   num_devices=NCORES)

    # per-core inputs
    d_obs = nc.dram_tensor("obs_packed", [128, R // 2], F32, kind="ExternalInput")
    d_qs = nc.dram_tensor("qs_row", [1, R], F32, kind="ExternalInput")
    # shared weights
    d_nodew2 = nc.dram_tensor("nodew2", [128, E], F32, kind="ExternalInput")
    d_nodew0 = nc.dram_tensor("nodew0", [1, E], F32, kind="ExternalInput")
    d_w1top = [nc.dram_tensor(f"w1top{l}", [E, E], F32, kind="ExternalInput") for l in range(L)]
    d_w1bot = [nc.dram_tensor(f"w1bot{l}", [E, H * E], F32, kind="ExternalInput") for l in range(L)]
    d_w2 = [nc.dram_tensor(f"w2_{l}", [E, E], F32, kind="ExternalInput") for l in range(L)]
    d_qpw1 = nc.dram_tensor("qpw1", [E, N * E], F32, kind="ExternalInput")
    d_qpw2 = nc.dram_tensor("qpw2", [E, 1], F32, kind="ExternalInput")
    d_blob = nc.dram_tensor("blob", [128, _BLOBC], F32, kind="ExternalInput")
    # output
    d_out = nc.dram_tensor("qout", [1, BC], F32, kind="ExternalOutput")

    with TileContext(nc) as tc, ExitStack() as ctx:
        wp = ctx.enter_context(tc.tile_pool(name="wp", bufs=1))
        big = ctx.enter_context(tc.tile_pool(name="big", bufs=1))
        sm = ctx.enter_context(tc.tile_pool(name="sm", bufs=2))
        wbdp = ctx.enter_context(tc.tile_pool(name="wbdp", bufs=4))
        ps512 = ctx.enter_context(tc.tile_pool(name="ps512", bufs=2, space="PSUM"))
        ps_hrm = ctx.enter_context(tc.tile_pool(name="ps_hrm", bufs=2, space="PSUM"))
        ps_msg = ctx.enter_context(tc.tile_pool(name="ps_msg", bufs=2, space="PSUM"))
        ps_misc = ctx.enter_context(tc.tile_pool(name="ps_misc", bufs=1, space="PSUM"))

        def load(name, dram, shape):
            t = wp.tile(shape, F32, name=name)
            nc.sync.dma_start(t[:, :], dram.ap())
            return t

        obsT = load("obsT", d_obs, [128, R // 2])
        qsr = load("qsr", d_qs, [1, R])
        nodew2 = load("nodew2_s", d_nodew2, [128, E])
        nodew0 = load("nodew0_s", d_nodew0, [1, E])
        w1top = [load(f"w1top{l}_s", d_w1top[l], [E, E]) for l in range(L)]
        w1bot = [load(f"w1bot{l}_s", d_w1bot[l], [E, H * E]) for l in range(L)]
        w2 = [load(f"w2_{l}_s", d_w2[l], [E, E]) for l in range(L)]
        qpw1 = load("qpw1_s", d_qpw1, [E, N * E])
        qpw2 = load("qpw2_s", d_qpw2, [E, 1])
        blob = load("blob_s", d_blob, [128, _BLOBC])

        I128 = blob[:, _BI:_BI + 128]
        bones = blob[:, _BONES:_BONES + 16]
        bonesT = blob[0:16, _BONEST:_BONEST + 128]
        mask64 = blob[:, _BMASK:_BMASK + 64]
        WjB = [blob[:, _BWJ0:_BWJ0 + 4], blob[:, _BWJ1:_BWJ1 + 4]]
        b1eff = [blob[:, _BB1E0:_BB1E0 + 1], blob[:, _BB1E1:_BB1E1 + 1]]
        qpb1 = blob[:, _BQPB1:_BQPB1 + 1]
        qpb2 = blob[0:1, _BQPB2:_BQPB2 + 1]

        ht_a = big.tile([128, R], F32, name="ht_a")
        ht_b = big.tile([128, R], F32, name="ht_b")
        hrm = big.tile([128, R], F32, name="hrm")
        uT = big.tile([128, R], F32, name="uT")
        msgT = big.tile([128, NCH * 64], F32, name="msgT")
        m2T = big.tile([128, BC], F32, name="m2T")

        # ---- node embedding: h0T = nodeW.T-form @ nfT (no bias) ----
        for c in range(8):
            ps = ps512.tile([128, 512], F32, name="node_ps", tag="spine")
            half = c // 4
            colo = (c % 4) * 512
            nc.tensor.matmul(ps[:, :], nodew2[64 * half:64 * half + 64, :],
                             obsT[64 * half:64 * half + 64, colo:colo + 512],
                             start=True, stop=False)
            nc.tensor.matmul(ps[:, :], nodew0[:, :],
                             qsr[:, c * 512:(c + 1) * 512],
                             start=False, stop=True)
            if c % 2 == 0:
                nc.vector.tensor_copy(ht_a[:, c * 512:(c + 1) * 512], ps[:, :])
            else:
                nc.scalar.activation(ht_a[:, c * 512:(c + 1) * 512], ps[:, :], AF.Copy)

        h_in, h_out = ht_a, ht_b
        for l in range(L):
            # ---- S5: transpose pass (+ sj) ----
            sj_ps = ps_misc.tile([128, 128], F32, name=f"sj_ps{l}", tag="sj")
            for c in range(NCH):
                hch = h_in[:, c * 128:(c + 1) * 128]
                hrm_ps = ps_hrm.tile([128, 128], F32, name="hrm_ps", tag="hrm")
                nc.tensor.matmul(hrm_ps[:, :], hch, I128, start=True, stop=True)
                nc.tensor.matmul(sj_ps[:, c * 4:(c + 1) * 4], hch, WjB[l],
                                 start=True, stop=True)
                if c % 2 == 0:
                    nc.vector.tensor_copy(hrm[:, c * 128:(c + 1) * 128], hrm_ps[:, :])
                else:
                    nc.scalar.activation(hrm[:, c * 128:(c + 1) * 128], hrm_ps[:, :], AF.Copy)
            # ---- softmax (batched over all chunks) ----
            exp_rm = sm.tile([128, 128], F32, name="exp_rm", tag="exp")
            nc.scalar.activation(exp_rm[:, :], sj_ps[:, :], AF.Exp)
            D_ps = ps_misc.tile([16, 128], F32, name="D_ps", tag="dd")
            nc.tensor.matmul(D_ps[:, :], bones, exp_rm[:, :], start=True, stop=True)
            Rr = sm.tile([16, 128], F32, name="Rr", tag="rr")
            nc.vector.reciprocal(Rr[:, :], D_ps[:, :])
            Rb_ps = ps_misc.tile([128, 128], F32, name="Rb_ps", tag="dd")
            nc.tensor.matmul(Rb_ps[:, :], bonesT, Rr[:, :], start=True, stop=True)
            w_rm = sm.tile([128, 128], F32, name="w_rm", tag="wrm")
            nc.vector.tensor_mul(w_rm[:, :], exp_rm[:, :], Rb_ps[:, :])
            # ---- wbd + msg matmuls ----
            for c in range(NCH):
                wbd = wbdp.tile([128, 64], F32, name="wbd", tag="wbd")
                wsl = w_rm[:, c * 4:(c + 1) * 4].unsqueeze(1).broadcast_to([128, GPC, 4])
                nc.vector.tensor_mul(wbd[:, :], wsl, mask64)
                msg_ps = ps_msg.tile([128, 64], F32, name="msg_ps", tag="msg")
                nc.tensor.matmul(msg_ps[:, :], hrm[:, c * 128:(c + 1) * 128],
                                 wbd[:, :], start=True, stop=True)
                if c % 2 == 0:
                    nc.scalar.activation(msgT[:, c * 64:(c + 1) * 64], msg_ps[:, :], AF.Copy)
                else:
                    nc.vector.tensor_copy(msgT[:, c * 64:(c + 1) * 64], msg_ps[:, :])
            # ---- m2T = sum_h W1bot_h.T-form @ msg_hT  (+ b1eff on drain) ----
            m2_ps = ps512.tile([128, 512], F32, name="m2_ps", tag="spine")
            msg4 = msgT.rearrange("p (c g h) -> p c g h", c=NCH, g=GPC, h=H)
            for h in range(H):
                nc.tensor.matmul(m2_ps[:, :], w1bot[l][:, h * E:(h + 1) * E],
                                 msg4[:, :, :, h:h + 1],
                                 start=(h == 0), stop=(h == H - 1))
            nc.scalar.activation(m2T[:, :], m2_ps[:, :], AF.Identity, bias=b1eff[l])
            # ---- A: t1 = W1top.T-form @ hT ; u = relu(t1 + m2 bcast) ----
            for c in range(8):
                t1_ps = ps512.tile([128, 512], F32, name="t1_ps", tag="spine")
                nc.tensor.matmul(t1_ps[:, :], w1top[l][:, :],
                                 h_in[:, c * 512:(c + 1) * 512], start=True, stop=True)
                m2v = m2T[:, c * 64:(c + 1) * 64].unsqueeze(2).broadcast_to([128, 64, N])
                usl = uT[:, c * 512:(c + 1) * 512]
                nc.vector.tensor_add(usl, t1_ps[:, :], m2v)
                nc.scalar.activation(usl, usl, AF.Relu)
            # ---- B: hT_new = hT + W2.T-form @ uT ----
            for c in range(8):
                d_ps = ps512.tile([128, 512], F32, name="d_ps", tag="spine")
                nc.tensor.matmul(d_ps[:, :], w2[l][:, :],
                                 uT[:, c * 512:(c + 1) * 512], start=True, stop=True)
                nc.vector.tensor_add(h_out[:, c * 512:(c + 1) * 512], d_ps[:, :],
                                     h_in[:, c * 512:(c + 1) * 512])
            h_in, h_out = h_out, h_in

        # ---- q predictor ----
        q1_ps = ps512.tile([128, 512], F32, name="q1_ps", tag="spine")
        hview = h_in.rearrange("p (b j) -> p b j", b=BC, j=N)
        for i in range(N):
            nc.tensor.matmul(q1_ps[:, :], qpw1[:, i * E:(i + 1) * E],
                             hview[:, :, i:i + 1], start=(i == 0), stop=(i == N - 1))
        q1 = sm.tile([128, BC], F32, name="q1", tag="q1")
        nc.scalar.activation(q1[:, :], q1_ps[:, :], AF.Relu, bias=qpb1)
        q2_ps = ps_misc.tile([1, BC], F32, name="q2_ps", tag="sj")
        nc.tensor.matmul(q2_ps[:, :], qpw2[:, :], q1[:, :], start=True, stop=True)
        qo = sm.tile([1, BC], F32, name="qo", tag="qo")
        nc.scalar.activation(qo[:, :], q2_ps[:, :], AF.Identity, bias=qpb2)
        nc.sync.dma_start(d_out.ap(), qo[:, :])

    nc.compile()
    return nc


_CACHE = {}


def _get_program():
    if "nc" not in _CACHE:
        _CACHE["nc"] = _build_program()
    return _CACHE["nc"]


def _prep_weights(node_W, node_b, attn_W, upd_W1, upd_b1, upd_W2, upd_b2,
                  qp_W1, qp_b1, qp_W2, qp_b2):
    f32 = np.float32
    Wj = [np.ascontiguousarray(attn_W[l, E:2 * E, :], f32) for l in range(L)]
    W1top = [np.ascontiguousarray(upd_W1[l, :E, :], f32) for l in range(L)]
    W1bot = [np.ascontiguousarray(upd_W1[l, E:, :], f32) for l in range(L)]
    W2 = [np.ascontiguousarray(upd_W2[l], f32) for l in range(L)]

    bcum = [node_b.astype(np.float64)]
    for l in range(L):
        bcum.append(bcum[-1] + upd_b2[l])
    b1eff = [
        (upd_b1[l] + bcum[l] @ W1top[l] + np.tile(bcum[l], H) @ W1bot[l]).astype(f32)
        for l in range(L)
    ]
    qp_b1eff = (qp_b1 + np.tile(bcum[L], N) @ qp_W1).astype(f32)

    w = {}
    w["nodew2"] = np.concatenate([node_W[1:], node_W[1:]], axis=0).astype(f32)
    w["nodew0"] = np.ascontiguousarray(node_W[0:1], f32)
    for l in range(L):
        w[f"w1top{l}"] = W1top[l]
        w[f"w1bot{l}"] = np.ascontiguousarray(
            W1bot[l].reshape(H, E, E).transpose(1, 0, 2).reshape(E, H * E), f32)
        w[f"w2_{l}"] = W2[l]
    w["qpw1"] = np.ascontiguousarray(
        qp_W1.reshape(N, E, E).transpose(1, 0, 2).reshape(E, N * E), f32)
    w["qpw2"] = np.ascontiguousarray(qp_W2, f32)

    blob = np.zeros((128, _BLOBC), f32)
    blob[:, _BI:_BI + 128] = np.eye(128, dtype=f32)
    r = np.arange(128)
    bones = (r[:, None] // N == np.arange(GPC)[None, :]).astype(f32)
    blob[:, _BONES:_BONES + 16] = bones
    blob[0:16, _BONEST:_BONEST + 128] = bones.T
    blob[:, _BMASK:_BMASK + 64] = np.repeat(bones, H, axis=1)
    blob[:, _BWJ0:_BWJ0 + 4] = Wj[0]
    blob[:, _BWJ1:_BWJ1 + 4] = Wj[1]
    blob[:, _BB1E0] = b1eff[0]
    blob[:, _BB1E1] = b1eff[1]
    blob[:, _BQPB1] = qp_b1eff
    blob[0, _BQPB2] = np.float32(qp_b2[0])
    w["blob"] = blob
    return w


def _ensure_ntff_hook():
    """Provide antenv.axon_hooks if the image lacks it (needed for trace=True).

    Replicates trn_agent_boot's ctypes NTFF hook against libaxon_pjrt.so.
    """
    try:
        from antenv.axon_hooks import get_axon_ntff_profile_hook  # noqa: F401
        return
    except ImportError:
        pass
    import contextlib
    import ctypes
    import types

    so_path = "/opt/axon/libaxon_pjrt.so"
    hook = None
    if os.path.exists(so_path):
        lib = ctypes.CDLL(so_path)
        if hasattr(lib, "axon_start_nrt_profile"):
            lib.axon_start_nrt_profile.argtypes = [
                ctypes.POINTER(ctypes.c_int64), ctypes.c_size_t]
            lib.axon_start_nrt_profile.restype = ctypes.c_int64
            lib.axon_stop_nrt_profile.argtypes = [ctypes.c_char_p]
            lib.axon_stop_nrt_profile.restype = ctypes.c_int64

            @contextlib.contextmanager
            def _hook(output_dir, device_ids):
                import jax
                jax.devices()
                if device_ids:
                    ids = (ctypes.c_int64 * len(device_ids))(*device_ids)
                    rc = lib.axon_start_nrt_profile(ids, len(device_ids))
                else:
                    rc = lib.axon_start_nrt_profile(None, 0)
                if rc != 0:
                    raise RuntimeError(f"axon_start_nrt_profile rc={rc}")
                try:
                    yield
                finally:
                    n = lib.axon_stop_nrt_profile(str(output_dir).encode())
                    print(f"profile: {n} file(s) -> {output_dir}", file=sys.stderr)

            hook = _hook

    mod = types.ModuleType("antenv.axon_hooks")
    _state = {"hook": hook}
    mod.get_axon_ntff_profile_hook = lambda: _state["hook"]
    mod.set_axon_ntff_profile_hook = lambda h: _state.update(hook=h)
    import antenv
    antenv.axon_hooks = mod
    sys.modules["antenv.axon_hooks"] = mod


def kernel(agent_qs, states, agent_obs, node_W, node_b, se_W1, se_b1, se_W2,
           se_b2, attn_W, attn_b, upd_W1, upd_b1, upd_W2, upd_b2,
           qp_W1, qp_b1, qp_W2, qp_b2):
    del states, se_W1, se_b1, se_W2, se_b2, attn_b  # cancel in softmax / unused
    nc = _get_program()
    w = _prep_weights(node_W, node_b, attn_W, upd_W1, upd_b1, upd_W2, upd_b2,
                      qp_W1, qp_b1, qp_W2, qp_b2)

    qs_flat = np.asarray(agent_qs, np.float32).reshape(B * N)
    obs_flat = np.asarray(agent_obs, np.float32).reshape(B * N, OBS)

    in_maps = []
    for c in range(NCORES):
        base = c * R
        o = obs_flat[base:base + R]                       # [4096, 64]
        packed = np.concatenate([o[:R // 2].T, o[R // 2:].T], axis=0)  # [128, 2048]
        m = dict(w)
        m["obs_packed"] = np.ascontiguousarray(packed)
        m["qs_row"] = np.ascontiguousarray(qs_flat[base:base + R][None, :])
        in_maps.append(m)

    trace = bool(int(os.environ.get("KBENCH_TRACE", "0")))
    if trace:
        _ensure_ntff_hook()
    res = bass_utils.run_bass_kernel_spmd(
        nc, in_maps, core_ids=list(range(NCORES)), trace=trace,
    )
    out = np.concatenate([res.results[c]["qout"].reshape(BC) for c in range(NCORES)])
    if res.exec_time_ns is not None:
        _CACHE["exec_time_ns"] = res.exec_time_ns
        _CACHE["results"] = res
    return out.reshape(BS, SEQ, 1).astype(np.float32)
